# revision 1
# baseline (speedup 1.0000x reference)
"""DeepseekV2 MLA attention kernel for Trainium2, 8-core tensor-parallel.

Strategy (per sharding hint): shard heads across the 8 cores (16 heads
each). q_b / kc / vc / o_w are sliced per-head on host; q_a and kv_a are
computed replicated on every core. Each core produces a partial [S, D]
output (its heads' contribution through o_proj); host sums the partials.

All device matmuls contract over the SBUF partition dimension, so the
whole computation is laid out "transposed" ([feature, seq]):
  hT [D, S]  ->  aT = qa_w^T @ hT  [1536, S]   (rmsnorm via ones-matmul
  partition sums of squares, rsqrt row broadcast by K=1 ones matmul)
  qT = qb^T @ aT_norm per head-pair, with per-pair column layout
  [nope(h0) nope(h1) pe(h0)|pe(h1) pe_rot(h0)|pe_rot(h1)], where pe_rot
  columns are pre-rotated/negated copies of pe columns so that RoPE is
  just  rope = pe*cosT + pe_rot*sinT  with no cross-partition moves.
  kvT likewise from kv_a_w extended with duplicated pe / pe_rot columns.
  scores^T[k,q] per head accumulate 5 matmuls (4x latent d-chunks + pe),
  exp on ACT (scale folded in; no max-subtraction needed -- logits are
  O(5)), causal mask via 0/1 mask multiply on the 8 diagonal tiles,
  softmax denominator via ones-column matmul, recip broadcast applied on
  the attn psum. out_v^T = vc^T @ attn^T; o_proj accumulates over the
  2048 local head*v dims into [S, D] partials.
"""
import sys
import math

sys.path.insert(0, '/opt/trn_rl_repo')

import numpy as np
import ml_dtypes
from contextlib import ExitStack

import concourse.bass as bass
import concourse.tile as tile
from concourse import bacc, mybir
from concourse.masks import make_identity

# ---- problem constants (hardcoded; kernel.py must be self-contained) ----
H = 128
D = 5120
Q_LORA = 1536
KV_LORA = 512
ROPE = 64
NOPE = 128
VDIM = 128
Q_HEAD = NOPE + ROPE
S = 1024
EPS = 1e-6
_MSCALE = 0.1 * 1.0 * math.log(40.0) + 1.0
SCALE = (Q_HEAD ** -0.5) * _MSCALE * _MSCALE

NCORES = 8
HLOC = H // NCORES          # 16 heads per core
QB_COLS = HLOC * (NOPE + 2 * ROPE)   # 4096
KV_COLS = KV_LORA + 4 * ROPE         # 512 latent + pe,pe,rot,rot = 768
OW_ROWS = HLOC * VDIM                # 2048

F32 = mybir.dt.float32
BF16 = mybir.dt.bfloat16

DK = D // 128          # 40 d-chunks
QK = Q_LORA // 128     # 12 q_lora chunks
SQ = S // 512          # 2 free-dim chunks of 512
SK = S // 128          # 8 key chunks of 128
LC = KV_LORA // 128    # 4 latent chunks
NPAIR = HLOC // 2      # 8 head pairs

bf16 = ml_dtypes.bfloat16


def build_program(reps=1, upto=3):
    nc = bacc.Bacc("TRN2", target_bir_lowering=False, debug=False,
                   num_devices=NCORES)

    hT_d = nc.dram_tensor("hT", [D, S], BF16, kind="ExternalInput").ap()
    cos2_d = nc.dram_tensor("cos2T", [128, S], BF16, kind="ExternalInput").ap()
    sin2_d = nc.dram_tensor("sin2T", [128, S], BF16, kind="ExternalInput").ap()
    qa_d = nc.dram_tensor("qa_w", [QK, 128, D], BF16, kind="ExternalInput").ap()
    qb_d = nc.dram_tensor("qb_w", [QB_COLS // 128, 128, Q_LORA], BF16,
                          kind="ExternalInput").ap()
    kvw_d = nc.dram_tensor("kv_w", [KV_COLS // 128, 128, D], BF16,
                           kind="ExternalInput").ap()
    kc_d = nc.dram_tensor("kc_w", [HLOC, NOPE, KV_LORA], BF16, kind="ExternalInput").ap()
    vc_d = nc.dram_tensor("vc_w", [HLOC, KV_LORA, VDIM], BF16, kind="ExternalInput").ap()
    ow_d = nc.dram_tensor("o_w", [OW_ROWS, D], BF16, kind="ExternalInput").ap()
    mask_d = nc.dram_tensor("masks", [4, 128, 512], BF16, kind="ExternalInput").ap()
    out_d = nc.dram_tensor("out", [S, D], F32, kind="ExternalOutput").ap()

    with tile.TileContext(nc) as tc, \
         nc.allow_low_precision(reason="float32r rows are fp32-width"):
      for _rep in range(reps):
       with ExitStack() as ctx:
        const = ctx.enter_context(tc.tile_pool(name="const", bufs=1))
        persist = ctx.enter_context(tc.tile_pool(name="persist", bufs=1))

        # ---- constants ----
        ident = const.tile([128, 128], BF16)
        make_identity(nc, ident)
        ones_col = const.tile([128, 1], BF16)   # lhsT for partition sums
        nc.vector.memset(ones_col, 1.0)
        eps_sb = const.tile([1, 1], F32)
        nc.vector.memset(eps_sb, EPS)
        cos2 = const.tile([128, S], BF16)
        nc.sync.dma_start(cos2, cos2_d)
        sin2 = const.tile([128, S], BF16)
        nc.sync.dma_start(sin2, sin2_d)
        masks = []
        for i in range(4):
            m_t = const.tile([128, 512], BF16, name=f"mask{i}")
            nc.sync.dma_start(m_t, mask_d[i])
            masks.append(m_t)

        # persistent activations (whole-program scope)
        a_sb = [persist.tile([128, S], BF16, name=f"a{m}") for m in range(QK)]
        lat_T = [persist.tile([128, S], BF16, name=f"latT{m}") for m in range(LC)]
        lat_kl = [persist.tile([128, KV_LORA], BF16, name=f"latkl{k}")
                  for k in range(SK)]
        kv_pe = persist.tile([128, S], BF16, name="kv_pe")

        # =========== phase 1: aT = qa^T @ hT, kvT = kvw^T @ hT ===========
        with tc.tile_pool(name="p1", bufs=2) as p1, \
             tc.tile_pool(name="p1s", bufs=4) as p1s:
            hT = []
            for k in range(DK):
                h_t = p1.tile([128, S], BF16, name=f"hT{k}", bufs=1)
                nc.sync.dma_start(h_t, hT_d[k * 128:(k + 1) * 128, :])
                hT.append(h_t)


            ss_a = p1s.tile([1, S], F32, bufs=1)
            nc.vector.memset(ss_a, 0.0)
            ss_kv = p1s.tile([1, S], F32, bufs=1)
            nc.vector.memset(ss_kv, 0.0)

            kv_lat_raw = [p1s.tile([128, S], BF16, name=f"kvraw{m}", bufs=1)
                          for m in range(LC)]
            kv_rot = p1s.tile([128, S], BF16, name="kv_rot", bufs=1)

            with tc.tile_pool(name="psA", bufs=1, space="PSUM") as psA:
                for m in range(QK + KV_COLS // 128):
                    is_q = m < QK
                    mm = m if is_q else m - QK
                    w_t = p1.tile([128, DK, 128], BF16, name="w_t",
                                  tag="w_stream")
                    src = qa_d if is_q else kvw_d
                    nc.sync.dma_start(
                        w_t, src[mm].rearrange("p (k c) -> p k c", c=128))
                    for qc in range(SQ):
                        acc = psA.tile([128, 512], F32, name="acc",
                                       tag="p1acc", bufs=3)
                        for k in range(DK):
                            nc.tensor.matmul(acc, w_t[:, k, :],
                                             hT[k][:, qc * 512:(qc + 1) * 512],
                                             start=(k == 0), stop=(k == DK - 1))
                        if is_q:
                            dst = a_sb[mm]
                        elif mm < LC:
                            dst = kv_lat_raw[mm]
                        elif mm == LC:
                            dst = kv_pe
                        else:
                            dst = kv_rot
                        nc.vector.tensor_copy(dst[:, qc * 512:(qc + 1) * 512], acc)
                        if is_q or mm < LC:
                            sq = p1.tile([128, 512], BF16, name="sq", tag="sq")
                            nc.scalar.square(sq, acc)
                            sqs = psA.tile([1, 512], F32, name="sqs",
                                           tag="sqs", bufs=2)
                            nc.tensor.matmul(sqs, ones_col, sq,
                                             start=True, stop=True)
                            tgt = ss_a if is_q else ss_kv
                            nc.vector.tensor_add(
                                tgt[:, qc * 512:(qc + 1) * 512],
                                tgt[:, qc * 512:(qc + 1) * 512], sqs)

            # rsqrt rows
            rstd_a = p1s.tile([1, S], F32, bufs=1)
            nc.scalar.activation(rstd_a, ss_a, mybir.ActivationFunctionType.Sqrt,
                                 bias=eps_sb, scale=1.0 / Q_LORA)
            nc.vector.reciprocal(rstd_a, rstd_a)
            rstd_kv = p1s.tile([1, S], F32, bufs=1)
            nc.scalar.activation(rstd_kv, ss_kv, mybir.ActivationFunctionType.Sqrt,
                                 bias=eps_sb, scale=1.0 / KV_LORA)
            nc.vector.reciprocal(rstd_kv, rstd_kv)

            for qc in range(SQ):
                sl = slice(qc * 512, (qc + 1) * 512)
                bc_a = p1s.tile([128, 512], F32, name="bc_a", tag="bc", bufs=2)
                nc.gpsimd.partition_broadcast(bc_a, rstd_a[:, sl])
                for m in range(QK):
                    nc.vector.tensor_mul(a_sb[m][:, sl], a_sb[m][:, sl], bc_a)
                bc_kv = p1s.tile([128, 512], F32, name="bc_kv", tag="bc", bufs=2)
                nc.gpsimd.partition_broadcast(bc_kv, rstd_kv[:, sl])
                for m in range(LC):
                    nc.vector.tensor_mul(lat_T[m][:, sl],
                                         kv_lat_raw[m][:, sl], bc_kv)

            # k_pe rope (pe duplicated in both partition halves by construction)
            nc.vector.tensor_mul(kv_pe, kv_pe, cos2)
            nc.vector.tensor_mul(kv_rot, kv_rot, sin2)
            nc.vector.tensor_add(kv_pe, kv_pe, kv_rot)

            # latent transpose -> [k, l] tiles
            with tc.tile_pool(name="psC", bufs=1, space="PSUM") as psC:
                for k in range(SK):
                    for lc in range(LC):
                        tp = psC.tile([128, 128], BF16, name="tp", tag="tp", bufs=4)
                        nc.tensor.transpose(
                            tp, lat_T[lc][:, k * 128:(k + 1) * 128], ident)
                        nc.vector.tensor_copy(
                            lat_kl[k][:, lc * 128:(lc + 1) * 128], tp)

        if upto < 2:
            with tc.tile_pool(name="anchor", bufs=1) as ap_, \
                 tc.tile_pool(name="psAn", bufs=1, space="PSUM") as psan:
                ob = ap_.tile([128, 512], F32, name="ob_anchor")
                nc.vector.tensor_copy(ob[:, 0:KV_LORA], lat_kl[0])
                nc.vector.tensor_copy(ob[:, 0:128], kv_pe[:, 0:128])
                nc.vector.tensor_copy(ob[:, 128:256], a_sb[QK - 1][:, 0:128])
                nc.sync.dma_start(out_d[0:128, 0:512], ob)
            continue

        # ====== phase 2+3: per head pair: q_b, rope, attention ======
        with tc.tile_pool(name="ov", bufs=1) as ovp:
            out_v = [ovp.tile([128, S], BF16, name=f"ov{h}") for h in range(HLOC)]

            with tc.tile_pool(name="ph", bufs=4) as ph, \
                 tc.tile_pool(name="pp", bufs=8) as pp, \
                 tc.tile_pool(name="psH", bufs=1, space="PSUM") as psH:
                for pr in range(NPAIR):
                    # --- q_b for this pair: 4 column chunks of 128 ---
                    q_nope = [ph.tile([128, S], BF16, name=f"qn{e}",
                                      tag=f"qn{e}", bufs=2) for e in (0, 1)]
                    q_pe = ph.tile([128, S], BF16, name="qpe", tag="qpe", bufs=2)
                    q_rot = ph.tile([128, S], BF16, name="qrot", tag="qrot", bufs=2)
                    dsts = [q_nope[0], q_nope[1], q_pe, q_rot]
                    for cc in range(4):
                        w_t = ph.tile([128, QK, 128], BF16, name="qb_t",
                                      tag="qb_stream", bufs=3)
                        nc.sync.dma_start(
                            w_t,
                            qb_d[pr * 4 + cc].rearrange("p (k c) -> p k c", c=128))
                        for qc in range(SQ):
                            sl = slice(qc * 512, (qc + 1) * 512)
                            acc = psH.tile([128, 512], F32, name="acc2",
                                           tag="hmm", bufs=2)
                            for k in range(QK):
                                nc.tensor.matmul(acc, w_t[:, k, :],
                                                 a_sb[k][:, sl],
                                                 start=(k == 0),
                                                 stop=(k == QK - 1))
                            nc.vector.tensor_copy(dsts[cc][:, sl], acc)
                    # rope: q_pe = q_pe*cos + q_rot*sin  (both heads at once)
                    nc.vector.tensor_mul(q_pe, q_pe, cos2)
                    nc.vector.tensor_mul(q_rot, q_rot, sin2)
                    nc.vector.tensor_add(q_pe, q_pe, q_rot)

                    kc_sbs, vc_sbs, qabs = [], [], []
                    for e in (0, 1):
                        h = 2 * pr + e
                        kc_sb = ph.tile([128, KV_LORA], BF16, name=f"kc{e}",
                                        tag=f"kc_w{e}")
                        nc.sync.dma_start(kc_sb, kc_d[h])
                        vc_sb = ph.tile([128, LC, VDIM], BF16, name=f"vc{e}",
                                        tag=f"vc_w{e}")
                        nc.sync.dma_start(
                            vc_sb, vc_d[h].rearrange("(lc p) v -> p lc v", p=128))
                        kc_sbs.append(kc_sb)
                        vc_sbs.append(vc_sb)
                        qabs.append([ph.tile([128, S], BF16, name=f"qabs{e}{lc}",
                                             tag=f"qabs{e}{lc}", bufs=2)
                                     for lc in range(LC)])

                    for e in (0, 1):
                        for lc in range(LC):
                            for qc in range(SQ):
                                sl = slice(qc * 512, (qc + 1) * 512)
                                acc = psH.tile([128, 512], F32, name="acckc",
                                               tag="hmm", bufs=2)
                                nc.tensor.matmul(
                                    acc, kc_sbs[e][:, lc * 128:(lc + 1) * 128],
                                    q_nope[e][:, sl], start=True, stop=True)
                                nc.vector.tensor_copy(qabs[e][lc][:, sl], acc)

                    for qc in range(SQ):
                        sl = slice(qc * 512, (qc + 1) * 512)
                        nkc = 4 if qc == 0 else SK
                        probs_all = {0: [], 1: []}
                        ssums = {}
                        atts = {}
                        for e in (0, 1):
                            ssums[e] = psH.tile([1, 512], F32, name=f"ssum{e}",
                                                tag=f"ssum{e}", bufs=1)
                            atts[e] = [psH.tile([128, 512], F32,
                                                name=f"att{e}{half}",
                                                tag=f"att{e}{half}", bufs=1)
                                       for half in (0, 1)]
                        # pass 0: scores + exp + mask + ssum + att lc 0,1
                        for kc in range(nkc):
                            ks = slice(kc * 128, (kc + 1) * 128)
                            for e in (0, 1):
                                pe_b = e * 64
                                sc = psH.tile([128, 512], F32, name="sc",
                                              tag="hmm", bufs=2)
                                for lc in range(LC):
                                    nc.tensor.matmul(sc, lat_T[lc][:, ks],
                                                     qabs[e][lc][:, sl],
                                                     start=(lc == 0), stop=False)
                                nc.tensor.matmul(
                                    sc, kv_pe[pe_b:pe_b + 64, ks],
                                    q_pe[pe_b:pe_b + 64, sl],
                                    start=False, stop=True)
                                probs = pp.tile([128, 512], BF16,
                                                name=f"probs{e}",
                                                tag=f"probs{e}", bufs=10)
                                nc.scalar.activation(
                                    probs, sc, mybir.ActivationFunctionType.Exp,
                                    scale=SCALE)
                                midx = kc - (0 if qc == 0 else 4)
                                if midx >= 0:
                                    nc.vector.tensor_mul(probs, probs, masks[midx])
                                probs_all[e].append(probs)
                                nc.tensor.matmul(ssums[e], ones_col, probs,
                                                 start=(kc == 0),
                                                 stop=(kc == nkc - 1))
                                for lc in (0, 1):
                                    nc.tensor.matmul(
                                        atts[e][lc],
                                        lat_kl[kc][:, lc * 128:(lc + 1) * 128],
                                        probs, start=(kc == 0),
                                        stop=(kc == nkc - 1))
                        # drain pass-0 att banks, then pass 1 reuses them
                        asb = {0: {}, 1: {}}
                        for e in (0, 1):
                            for lc in (0, 1):
                                t = pp.tile([128, 512], BF16, name=f"asb{e}",
                                            tag=f"asb{e}", bufs=5)
                                nc.vector.tensor_copy(t, atts[e][lc])
                                asb[e][lc] = t
                        for e in (0, 1):
                            atts[e] = [psH.tile([128, 512], F32,
                                                name=f"att{e}{half}b",
                                                tag=f"att{e}{half}", bufs=1)
                                       for half in (0, 1)]
                        for kc in range(nkc):
                            for e in (0, 1):
                                for i, lc in enumerate((2, 3)):
                                    nc.tensor.matmul(
                                        atts[e][i],
                                        lat_kl[kc][:, lc * 128:(lc + 1) * 128],
                                        probs_all[e][kc], start=(kc == 0),
                                        stop=(kc == nkc - 1))
                        for e in (0, 1):
                            h = 2 * pr + e
                            recip = ph.tile([1, 512], F32, name=f"recip{e}",
                                            tag=f"recip{e}", bufs=2)
                            nc.vector.reciprocal(recip, ssums[e])
                            bc_sb = pp.tile([128, 512], F32, name=f"bc_sb{e}",
                                            tag=f"bcsb{e}", bufs=2)
                            nc.gpsimd.partition_broadcast(bc_sb, recip)
                            for i, lc in enumerate((2, 3)):
                                t = pp.tile([128, 512], BF16, name=f"asb{e}b",
                                            tag=f"asb{e}", bufs=5)
                                nc.vector.tensor_copy(t, atts[e][i])
                                asb[e][lc] = t
                            vout = psH.tile([128, 512], F32, name=f"vout{e}",
                                            tag="hmm", bufs=2)
                            for lc in range(LC):
                                nc.tensor.matmul(vout, vc_sbs[e][:, lc, :],
                                                 asb[e][lc],
                                                 start=(lc == 0),
                                                 stop=(lc == LC - 1))
                            nc.vector.tensor_mul(out_v[h][:, sl], vout, bc_sb)

            # =========== phase 6: o_proj partials ===========
            with tc.tile_pool(name="p6", bufs=6) as p6, \
                 tc.tile_pool(name="ps6", bufs=1, space="PSUM") as ps6:
                for dc in range(D // 512):
                    po = [ps6.tile([128, 512], F32, name=f"po{sc2}",
                                   tag=f"po{sc2}", bufs=1)
                          for sc2 in range(SK)]
                    for hv in range(HLOC):
                        ow_t = p6.tile([128, 512], BF16, name="ow_t",
                                       tag="ow_stream")
                        nc.sync.dma_start(
                            ow_t,
                            ow_d[hv * 128:(hv + 1) * 128, dc * 512:(dc + 1) * 512])
                        for sc2 in range(SK):
                            nc.tensor.matmul(
                                po[sc2], out_v[hv][:, sc2 * 128:(sc2 + 1) * 128],
                                ow_t, start=(hv == 0), stop=(hv == HLOC - 1))
                    for sc2 in range(SK):
                        ob = p6.tile([128, 512], F32, name="ob", tag="ob")
                        nc.scalar.copy(ob, po[sc2])
                        nc.sync.dma_start(
                            out_d[sc2 * 128:(sc2 + 1) * 128,
                                  dc * 512:(dc + 1) * 512], ob)

    nc.compile()
    return nc


def host_prep(hidden_states, cos, sin, q_a_w, q_a_ln_w, q_b_w, kv_a_w,
              kv_a_ln_w, kc_w, vc_w, o_w):
    """Shard + preprocess full fp32 inputs into per-core bf16 in_maps."""
    h = np.asarray(hidden_states)[0]            # [S, D]
    hT = np.ascontiguousarray(h.T).astype(bf16)  # [D, S]
    cosT = np.ascontiguousarray(np.asarray(cos).T)  # [64, S]
    sinT = np.ascontiguousarray(np.asarray(sin).T)
    cos2T = np.concatenate([cosT, cosT], 0).astype(bf16)   # [128, S]
    sin2T = np.concatenate([sinT, sinT], 0).astype(bf16)

    qa0 = np.asarray(q_a_w).astype(bf16)         # [D, 1536]
    qa = np.ascontiguousarray(
        qa0.reshape(DK, 128, QK, 128).transpose(2, 1, 0, 3).reshape(QK, 128, D))
    qb = (np.asarray(q_b_w) * np.asarray(q_a_ln_w)[:, None])  # ln folded

    lat_w = np.asarray(kv_a_w)[:, :KV_LORA]
    pe_w = np.asarray(kv_a_w)[:, KV_LORA:]
    rot_w = np.concatenate([-pe_w[:, ROPE // 2:], pe_w[:, :ROPE // 2]], 1)
    kvw0 = np.concatenate([lat_w, pe_w, pe_w, rot_w, rot_w], 1).astype(bf16)
    nkv = KV_COLS // 128
    kvw = np.ascontiguousarray(
        kvw0.reshape(DK, 128, nkv, 128).transpose(2, 1, 0, 3).reshape(nkv, 128, D))

    kv_ln = np.asarray(kv_a_ln_w)
    kc_eff = (np.asarray(kc_w) * kv_ln[None, None, :]).astype(bf16)  # [H,128,512]
    vc_eff = (np.asarray(vc_w) * kv_ln[None, :, None]).astype(bf16)  # [H,512,128]
    ow = np.asarray(o_w)

    i = np.arange(128)[:, None]
    j = np.arange(512)[None, :]
    masks = np.stack([(j >= i + off).astype(np.float32)
                      for off in (0, 128, 256, 384)]).astype(bf16)

    in_maps = []
    for c in range(NCORES):
        blocks = []
        for p in range(NPAIR):
            h0 = c * HLOC + 2 * p
            h1 = h0 + 1
            n0 = qb[:, h0 * Q_HEAD: h0 * Q_HEAD + NOPE]
            n1 = qb[:, h1 * Q_HEAD: h1 * Q_HEAD + NOPE]
            p0 = qb[:, h0 * Q_HEAD + NOPE: (h0 + 1) * Q_HEAD]
            p1 = qb[:, h1 * Q_HEAD + NOPE: (h1 + 1) * Q_HEAD]
            r0 = np.concatenate([-p0[:, ROPE // 2:], p0[:, :ROPE // 2]], 1)
            r1 = np.concatenate([-p1[:, ROPE // 2:], p1[:, :ROPE // 2]], 1)
            blocks += [n0, n1, np.concatenate([p0, p1], 1),
                       np.concatenate([r0, r1], 1)]
        qb_core0 = np.concatenate(blocks, axis=1).astype(bf16)  # [1536, 4096]
        nqb = QB_COLS // 128
        qb_core = np.ascontiguousarray(
            qb_core0.reshape(QK, 128, nqb, 128).transpose(2, 1, 0, 3)
            .reshape(nqb, 128, Q_LORA))
        in_maps.append({
            "hT": hT,
            "cos2T": cos2T,
            "sin2T": sin2T,
            "qa_w": qa,
            "qb_w": qb_core,
            "kv_w": kvw,
            "kc_w": kc_eff[c * HLOC:(c + 1) * HLOC],
            "vc_w": vc_eff[c * HLOC:(c + 1) * HLOC],
            "o_w": ow[c * OW_ROWS:(c + 1) * OW_ROWS].astype(bf16),
            "masks": masks,
        })
    return in_maps


_CACHE = {}


def _get_program():
    if "nc" not in _CACHE:
        _CACHE["nc"] = build_program()
    return _CACHE["nc"]


def kernel(**inputs) -> np.ndarray:
    from concourse.bass_utils import run_bass_kernel_spmd
    nc = _get_program()
    in_maps = host_prep(**inputs)
    res = run_bass_kernel_spmd(nc, in_maps, list(range(NCORES)))
    out = np.zeros((S, D), np.float64)
    for c in range(NCORES):
        out += res.results[c]["out"].astype(np.float64)
    return out.astype(np.float32)[None]



# revision 9
# speedup vs baseline: 1.3413x; 1.3413x over previous
"""DeepseekV2 MLA attention kernel for Trainium2, 8-core tensor-parallel.

Strategy: heads sharded across 8 cores (16 each) as before, but the
replicated q_a/kv_a projections are now *column-sharded* across cores
and reassembled with collectives:

  W_comb [D, 2048] = [q_a (12 chunks) | kv latent (4 chunks)].  Core c
  computes chunks 2c, 2c+1 of aT_raw/latT_raw for each seq half (free
  512), contributes per-half partial sum-of-squares stats (pre-scaled by
  1/N via a per-core selector matrix) to an AllReduce, and the two raw
  chunks to an AllGather.  The 64-dim k_pe projection (duplicated into
  128 rows) is *seq-sharded*: each core computes all 128 pe rows for its
  128 seq positions, AllGathered separately.  RMSNorm is applied locally
  after the collectives: rstd rows from the reduced stats, broadcast to
  128 partitions via a ones-column PE matmul; the k-major latent copy is
  produced by DMA-transpose from the AllGather output and normalized
  with a per-partition tensor_scalar multiply.

  RoPE rotate-half no longer consumes extra projection columns: a
  constant 128x128 signed permutation matrix rot_pt gives
  rot(x) = rot_pt^T @ x as one PE matmul per 512 columns, for both q_pe
  (per head pair) and k_pe.

  Attention (scores / softmax / probs @ latent / vc / o_proj) is
  unchanged from the head-sharded baseline: scores^T accumulate latent +
  pe matmuls per 128-key chunk, exp on ACT with the softmax scale folded
  in, causal mask by 0/1 multiply on diagonal tiles, denominator via
  ones-column matmul, o_proj accumulates the local 2048 head*v dims into
  a full [S, D] partial that the host sums across cores.
"""
import sys
import math

sys.path.insert(0, '/opt/trn_rl_repo')

import numpy as np
import ml_dtypes
from contextlib import ExitStack

import concourse.bass as bass
import concourse.tile as tile
from concourse import bacc, mybir
from concourse.masks import make_identity

# ---- problem constants (hardcoded; kernel.py must be self-contained) ----
H = 128
D = 5120
Q_LORA = 1536
KV_LORA = 512
ROPE = 64
NOPE = 128
VDIM = 128
Q_HEAD = NOPE + ROPE
S = 1024
EPS = 1e-6
_MSCALE = 0.1 * 1.0 * math.log(40.0) + 1.0
SCALE = (Q_HEAD ** -0.5) * _MSCALE * _MSCALE

NCORES = 8
HLOC = H // NCORES          # 16 heads per core
QB_COLS = HLOC * (NOPE + ROPE)       # 3072 (no rot block)
OW_ROWS = HLOC * VDIM                # 2048

F32 = mybir.dt.float32
BF16 = mybir.dt.bfloat16

DK = D // 128          # 40 d-chunks
QK = Q_LORA // 128     # 12 q_lora chunks
SQ = S // 512          # 2 free-dim chunks of 512
SK = S // 128          # 8 key chunks of 128
LC = KV_LORA // 128    # 4 latent chunks
NPAIR = HLOC // 2      # 8 head pairs
WCH = 16               # chunks in combined projection = QK + LC
WLOC = WCH // NCORES   # 2 chunks per core

bf16 = ml_dtypes.bfloat16

GROUPS = [list(range(NCORES))]


def build_program(reps=1, upto=3):
    nc = bacc.Bacc("TRN2", target_bir_lowering=False, debug=False,
                   num_devices=NCORES)

    hT_d = nc.dram_tensor("hT", [D, S], BF16, kind="ExternalInput").ap()
    cos2_d = nc.dram_tensor("cos2T", [128, S], BF16, kind="ExternalInput").ap()
    sin2_d = nc.dram_tensor("sin2T", [128, S], BF16, kind="ExternalInput").ap()
    wc_d = nc.dram_tensor("wc_w", [WLOC, 128, D], BF16, kind="ExternalInput").ap()
    sel2_d = nc.dram_tensor("sel2", [128, 2], BF16, kind="ExternalInput").ap()
    hpe_d = nc.dram_tensor("hpe", [128, DK, 128], BF16, kind="ExternalInput").ap()
    pew_d = nc.dram_tensor("pew", [128, DK, 128], BF16, kind="ExternalInput").ap()
    rotp_d = nc.dram_tensor("rot_pt", [128, 128], BF16, kind="ExternalInput").ap()
    qb_d = nc.dram_tensor("qb_w", [QB_COLS // 128, 128, Q_LORA], BF16,
                          kind="ExternalInput").ap()
    kc_d = nc.dram_tensor("kc_w", [HLOC, NOPE, KV_LORA], BF16, kind="ExternalInput").ap()
    vc_d = nc.dram_tensor("vc_w", [HLOC, KV_LORA, VDIM], BF16, kind="ExternalInput").ap()
    ow_d = nc.dram_tensor("o_w", [OW_ROWS, D], BF16, kind="ExternalInput").ap()
    mask_d = nc.dram_tensor("masks", [4, 128, 512], BF16, kind="ExternalInput").ap()
    out_d = nc.dram_tensor("out", [S, D], F32, kind="ExternalOutput").ap()

    with tile.TileContext(nc) as tc, \
         nc.allow_low_precision(reason="float32r rows are fp32-width"):
      for _rep in range(reps):
       with ExitStack() as ctx:
        const = ctx.enter_context(tc.tile_pool(name="const", bufs=1))
        persist = ctx.enter_context(tc.tile_pool(name="persist", bufs=1))

        # DRAM bounce buffers for collectives (fresh names per rep)
        ag1_in = [nc.dram_tensor(f"ag1i{_rep}_{h}", [WLOC, 128, 512],
                                 BF16).ap() for h in range(SQ)]
        ag1_out = [nc.dram_tensor(f"ag1o{_rep}_{h}", [WCH, 128, 512],
                                  BF16).ap() for h in range(SQ)]
        ar_in = [nc.dram_tensor(f"ari{_rep}_{h}", [2, 512], F32).ap()
                 for h in range(SQ)]
        ar_out = [nc.dram_tensor(f"aro{_rep}_{h}", [2, 512], F32).ap()
                  for h in range(SQ)]
        ag2_in = nc.dram_tensor(f"ag2i{_rep}", [128, 128], BF16).ap()
        ag2_out = nc.dram_tensor(f"ag2o{_rep}", [NCORES * 128, 128], BF16).ap()

        # ---- constants ----
        ident = const.tile([128, 128], BF16)
        make_identity(nc, ident)
        identf = const.tile([128, 128], F32)
        make_identity(nc, identf)
        ones_col = const.tile([128, 1], BF16)   # lhsT for partition sums
        nc.vector.memset(ones_col, 1.0)
        ones1f = const.tile([1, 128], F32)      # lhsT for row->128 broadcast
        nc.vector.memset(ones1f, 1.0)
        eps_sb = const.tile([1, 1], F32)
        nc.vector.memset(eps_sb, EPS)
        cos2 = const.tile([128, S], BF16)
        nc.sync.dma_start(cos2, cos2_d)
        sin2 = const.tile([128, S], BF16)
        nc.sync.dma_start(sin2, sin2_d)
        rotp = const.tile([128, 128], BF16)
        nc.sync.dma_start(rotp, rotp_d)
        sel2 = const.tile([128, 2], BF16)
        nc.sync.dma_start(sel2, sel2_d)
        masks = []
        for i in range(4):
            m_t = const.tile([128, 512], BF16, name=f"mask{i}")
            nc.sync.dma_start(m_t, mask_d[i])
            masks.append(m_t)

        # persistent activations (whole-program scope)
        a_sb = [persist.tile([128, S], BF16, name=f"a{m}") for m in range(QK)]
        lat_T = [persist.tile([128, S], BF16, name=f"latT{m}") for m in range(LC)]
        lat_kl = [persist.tile([128, KV_LORA], BF16, name=f"latkl{k}")
                  for k in range(SK)]
        kv_pe = persist.tile([128, S], BF16, name="kv_pe")

        # ====== phase 1: sharded [q_a | kv_a] projection + collectives ======
        with tc.tile_pool(name="p1", bufs=2) as p1, \
             tc.tile_pool(name="p1s", bufs=4) as p1s:
            hT = []
            for k in range(DK):
                h_t = p1.tile([128, S], BF16, name=f"hT{k}", bufs=1)
                nc.sync.dma_start(h_t, hT_d[k * 128:(k + 1) * 128, :])
                hT.append(h_t)
            wc = []
            for m in range(WLOC):
                w_t = p1.tile([128, DK, 128], BF16, name=f"wc{m}", bufs=1)
                nc.sync.dma_start(
                    w_t, wc_d[m].rearrange("p (k c) -> p k c", c=128))
                wc.append(w_t)
            hpe = p1.tile([128, DK, 128], BF16, name="hpe", bufs=1)
            nc.sync.dma_start(hpe, hpe_d)
            pew = p1.tile([128, DK, 128], BF16, name="pew", bufs=1)
            nc.sync.dma_start(pew, pew_d)

            with tc.tile_pool(name="psA", bufs=1, space="PSUM") as psA:
                for qc in range(SQ):
                    sl = slice(qc * 512, (qc + 1) * 512)
                    stats_ps = psA.tile([2, 512], F32, name="stats",
                                        tag="stats", bufs=2)
                    for m in range(WLOC):
                        acc = psA.tile([128, 512], F32, name="acc",
                                       tag="p1acc", bufs=3)
                        for k in range(DK):
                            nc.tensor.matmul(acc, wc[m][:, k, :],
                                             hT[k][:, sl],
                                             start=(k == 0), stop=(k == DK - 1))
                        contrib = p1.tile([128, 512], BF16, name="contrib",
                                          tag="contrib", bufs=4)
                        nc.vector.tensor_copy(contrib, acc)
                        nc.sync.dma_start(ag1_in[qc][m], contrib)
                        sq = p1.tile([128, 512], BF16, name="sq", tag="sq")
                        nc.scalar.square(sq, acc)
                        nc.tensor.matmul(stats_ps, sel2, sq,
                                         start=(m == 0), stop=(m == WLOC - 1))
                    stats_sb = p1s.tile([2, 512], F32, name="stats_sb",
                                        tag="stats_sb", bufs=2)
                    nc.scalar.copy(stats_sb, stats_ps)
                    nc.sync.dma_start(ar_in[qc], stats_sb)
                    nc.gpsimd.collective_compute(
                        "AllReduce", mybir.AluOpType.add,
                        replica_groups=GROUPS,
                        ins=[ar_in[qc]], outs=[ar_out[qc]])
                    nc.gpsimd.collective_compute(
                        "AllGather", mybir.AluOpType.bypass,
                        replica_groups=GROUPS,
                        ins=[ag1_in[qc]], outs=[ag1_out[qc]])

                # seq-sharded k_pe projection (128 dup rows x my 128 cols)
                pe_ps = psA.tile([128, 128], F32, name="pe_ps", tag="stats",
                                 bufs=2)
                for k in range(DK):
                    nc.tensor.matmul(pe_ps, pew[:, k, :], hpe[:, k, :],
                                     start=(k == 0), stop=(k == DK - 1))
                pe_sb = p1s.tile([128, 128], BF16, name="pe_sb", bufs=1)
                nc.scalar.copy(pe_sb, pe_ps)
                nc.sync.dma_start(ag2_in, pe_sb)
                nc.gpsimd.collective_compute(
                    "AllGather", mybir.AluOpType.bypass,
                    replica_groups=GROUPS,
                    ins=[ag2_in], outs=[ag2_out])

            # ---- phase 1b: unpack + normalize ----
            kvpe_raw = p1s.tile([128, S], BF16, name="kvpe_raw", bufs=1)
            with tc.tile_pool(name="psB", bufs=1, space="PSUM") as psB:
                for qc in range(SQ):
                    sl = slice(qc * 512, (qc + 1) * 512)
                    st_a = p1s.tile([1, 512], F32, name="st_a", tag="st_a",
                                    bufs=2)
                    nc.sync.dma_start(st_a, ar_out[qc][0:1, :])
                    st_kv = p1s.tile([1, 512], F32, name="st_kv", tag="st_kv",
                                     bufs=2)
                    nc.sync.dma_start(st_kv, ar_out[qc][1:2, :])
                    rstd_a = p1s.tile([1, 512], F32, name="rstd_a",
                                      tag="rstd_a", bufs=2)
                    nc.scalar.activation(
                        rstd_a, st_a, mybir.ActivationFunctionType.Sqrt,
                        bias=eps_sb, scale=1.0)
                    nc.vector.reciprocal(rstd_a, rstd_a)
                    rstd_kv = p1s.tile([1, 512], F32, name="rstd_kv",
                                       tag="rstd_kv", bufs=2)
                    nc.scalar.activation(
                        rstd_kv, st_kv, mybir.ActivationFunctionType.Sqrt,
                        bias=eps_sb, scale=1.0)
                    nc.vector.reciprocal(rstd_kv, rstd_kv)

                    # broadcast rstd rows to 128 partitions on PE
                    bc_a = psB.tile([128, 512], F32, name="bc_a", tag="bc_a",
                                    bufs=2)
                    nc.tensor.matmul(bc_a, ones1f, rstd_a,
                                     start=True, stop=True)
                    bc_kv = psB.tile([128, 512], F32, name="bc_kv",
                                     tag="bc_kv", bufs=2)
                    nc.tensor.matmul(bc_kv, ones1f, rstd_kv,
                                     start=True, stop=True)

                    for m in range(QK):
                        nc.sync.dma_start(a_sb[m][:, sl], ag1_out[qc][m])
                        nc.vector.tensor_mul(a_sb[m][:, sl], a_sb[m][:, sl],
                                             bc_a)
                    for m in range(LC):
                        nc.sync.dma_start(lat_T[m][:, sl],
                                          ag1_out[qc][QK + m])
                        nc.vector.tensor_mul(lat_T[m][:, sl],
                                             lat_T[m][:, sl], bc_kv)

                    # k-major latent via DMA transpose + per-partition scale
                    lat_flat = ag1_out[qc].rearrange("m p f -> (m p) f")
                    for kk in range(4):
                        k = qc * 4 + kk
                        nc.sync.dma_start_transpose(
                            lat_kl[k],
                            lat_flat[QK * 128:, kk * 128:(kk + 1) * 128])
                        rt_ps = psB.tile([128, 1], F32, name="rt",
                                         tag="rt", bufs=2)
                        nc.tensor.transpose(
                            rt_ps, rstd_kv[:, kk * 128:(kk + 1) * 128],
                            identf[0:1, 0:1])
                        rt_sb = p1s.tile([128, 1], F32, name="rt_sb",
                                         tag="rt_sb", bufs=4)
                        nc.scalar.copy(rt_sb, rt_ps)
                        nc.vector.tensor_scalar_mul(lat_kl[k], lat_kl[k],
                                                    rt_sb)

                # k_pe: regather, rotate (PE), rope (DVE)
                nc.sync.dma_start(
                    kvpe_raw.rearrange("p (c f) -> p c f", f=128),
                    ag2_out.rearrange("(c p) f -> p c f", p=128))
                for qc in range(SQ):
                    sl = slice(qc * 512, (qc + 1) * 512)
                    rot_ps = psB.tile([128, 512], F32, name="rot_ps",
                                      tag="bc_a", bufs=2)
                    nc.tensor.matmul(rot_ps, rotp, kvpe_raw[:, sl],
                                     start=True, stop=True)
                    rot_sb = p1s.tile([128, 512], BF16, name="rot_sb",
                                      tag="rot_sb", bufs=2)
                    nc.vector.tensor_copy(rot_sb, rot_ps)
                    nc.vector.tensor_mul(rot_sb, rot_sb, sin2[:, sl])
                    nc.vector.tensor_mul(kv_pe[:, sl], kvpe_raw[:, sl],
                                         cos2[:, sl])
                    nc.vector.tensor_add(kv_pe[:, sl], kv_pe[:, sl], rot_sb)

        if upto < 2:
            with tc.tile_pool(name="anchor", bufs=1) as ap_:
                ob = ap_.tile([128, 512], F32, name="ob_anchor")
                nc.vector.tensor_copy(ob[:, 0:KV_LORA], lat_kl[0])
                nc.vector.tensor_copy(ob[:, 0:128], kv_pe[:, 0:128])
                nc.vector.tensor_copy(ob[:, 128:256], a_sb[QK - 1][:, 0:128])
                nc.sync.dma_start(out_d[0:128, 0:512], ob)
            continue

        # ====== phase 2+3: per head pair: q_b, rope, attention ======
        with tc.tile_pool(name="ov", bufs=1) as ovp:
            out_v = [ovp.tile([128, S], BF16, name=f"ov{h}") for h in range(HLOC)]

            with tc.tile_pool(name="ph", bufs=4) as ph, \
                 tc.tile_pool(name="pp", bufs=8) as pp, \
                 tc.tile_pool(name="psH", bufs=1, space="PSUM") as psH:
                for pr in range(NPAIR):
                    # --- q_b for this pair: 3 column chunks of 128 ---
                    q_nope = [ph.tile([128, S], BF16, name=f"qn{e}",
                                      tag=f"qn{e}", bufs=2) for e in (0, 1)]
                    q_pe = ph.tile([128, S], BF16, name="qpe", tag="qpe", bufs=2)
                    dsts = [q_nope[0], q_nope[1], q_pe]
                    for cc in range(3):
                        w_t = ph.tile([128, QK, 128], BF16, name="qb_t",
                                      tag="qb_stream", bufs=3)
                        nc.sync.dma_start(
                            w_t,
                            qb_d[pr * 3 + cc].rearrange("p (k c) -> p k c", c=128))
                        for qc in range(SQ):
                            sl = slice(qc * 512, (qc + 1) * 512)
                            acc = psH.tile([128, 512], F32, name="acc2",
                                           tag="hmm", bufs=2)
                            for k in range(QK):
                                nc.tensor.matmul(acc, w_t[:, k, :],
                                                 a_sb[k][:, sl],
                                                 start=(k == 0),
                                                 stop=(k == QK - 1))
                            nc.vector.tensor_copy(dsts[cc][:, sl], acc)
                    # rope: q_pe = q_pe*cos + rot(q_pe)*sin  (both heads at once)
                    for qc in range(SQ):
                        sl = slice(qc * 512, (qc + 1) * 512)
                        rot_ps = psH.tile([128, 512], F32, name="qrot_ps",
                                          tag="hmm", bufs=2)
                        nc.tensor.matmul(rot_ps, rotp, q_pe[:, sl],
                                         start=True, stop=True)
                        q_rot = ph.tile([128, 512], BF16, name="qrot",
                                        tag="qrot", bufs=2)
                        nc.vector.tensor_copy(q_rot, rot_ps)
                        nc.vector.tensor_mul(q_rot, q_rot, sin2[:, sl])
                        nc.vector.tensor_mul(q_pe[:, sl], q_pe[:, sl],
                                             cos2[:, sl])
                        nc.vector.tensor_add(q_pe[:, sl], q_pe[:, sl], q_rot)

                    kc_sbs, vc_sbs, qabs = [], [], []
                    for e in (0, 1):
                        h = 2 * pr + e
                        kc_sb = ph.tile([128, KV_LORA], BF16, name=f"kc{e}",
                                        tag=f"kc_w{e}")
                        nc.sync.dma_start(kc_sb, kc_d[h])
                        vc_sb = ph.tile([128, LC, VDIM], BF16, name=f"vc{e}",
                                        tag=f"vc_w{e}")
                        nc.sync.dma_start(
                            vc_sb, vc_d[h].rearrange("(lc p) v -> p lc v", p=128))
                        kc_sbs.append(kc_sb)
                        vc_sbs.append(vc_sb)
                        qabs.append([ph.tile([128, S], BF16, name=f"qabs{e}{lc}",
                                             tag=f"qabs{e}{lc}", bufs=2)
                                     for lc in range(LC)])

                    for e in (0, 1):
                        for lc in range(LC):
                            for qc in range(SQ):
                                sl = slice(qc * 512, (qc + 1) * 512)
                                acc = psH.tile([128, 512], F32, name="acckc",
                                               tag="hmm", bufs=2)
                                nc.tensor.matmul(
                                    acc, kc_sbs[e][:, lc * 128:(lc + 1) * 128],
                                    q_nope[e][:, sl], start=True, stop=True)
                                nc.vector.tensor_copy(qabs[e][lc][:, sl], acc)

                    for qc in range(SQ):
                        sl = slice(qc * 512, (qc + 1) * 512)
                        nkc = 4 if qc == 0 else SK
                        probs_all = {0: [], 1: []}
                        ssums = {}
                        atts = {}
                        for e in (0, 1):
                            ssums[e] = psH.tile([1, 512], F32, name=f"ssum{e}",
                                                tag=f"ssum{e}", bufs=1)
                            atts[e] = [psH.tile([128, 512], F32,
                                                name=f"att{e}{half}",
                                                tag=f"att{e}{half}", bufs=1)
                                       for half in (0, 1)]
                        # pass 0: scores + exp + mask + ssum + att lc 0,1
                        for kc in range(nkc):
                            ks = slice(kc * 128, (kc + 1) * 128)
                            for e in (0, 1):
                                pe_b = e * 64
                                sc = psH.tile([128, 512], F32, name="sc",
                                              tag="hmm", bufs=2)
                                for lc in range(LC):
                                    nc.tensor.matmul(sc, lat_T[lc][:, ks],
                                                     qabs[e][lc][:, sl],
                                                     start=(lc == 0), stop=False)
                                nc.tensor.matmul(
                                    sc, kv_pe[pe_b:pe_b + 64, ks],
                                    q_pe[pe_b:pe_b + 64, sl],
                                    start=False, stop=True)
                                probs = pp.tile([128, 512], BF16,
                                                name=f"probs{e}",
                                                tag=f"probs{e}", bufs=10)
                                nc.scalar.activation(
                                    probs, sc, mybir.ActivationFunctionType.Exp,
                                    scale=SCALE)
                                midx = kc - (0 if qc == 0 else 4)
                                if midx >= 0:
                                    nc.vector.tensor_mul(probs, probs, masks[midx])
                                probs_all[e].append(probs)
                                nc.tensor.matmul(ssums[e], ones_col, probs,
                                                 start=(kc == 0),
                                                 stop=(kc == nkc - 1))
                                for lc in (0, 1):
                                    nc.tensor.matmul(
                                        atts[e][lc],
                                        lat_kl[kc][:, lc * 128:(lc + 1) * 128],
                                        probs, start=(kc == 0),
                                        stop=(kc == nkc - 1))
                        # drain pass-0 att banks, then pass 1 reuses them
                        asb = {0: {}, 1: {}}
                        for e in (0, 1):
                            for lc in (0, 1):
                                t = pp.tile([128, 512], BF16, name=f"asb{e}",
                                            tag=f"asb{e}", bufs=5)
                                nc.vector.tensor_copy(t, atts[e][lc])
                                asb[e][lc] = t
                        for e in (0, 1):
                            atts[e] = [psH.tile([128, 512], F32,
                                                name=f"att{e}{half}b",
                                                tag=f"att{e}{half}", bufs=1)
                                       for half in (0, 1)]
                        for kc in range(nkc):
                            for e in (0, 1):
                                for i, lc in enumerate((2, 3)):
                                    nc.tensor.matmul(
                                        atts[e][i],
                                        lat_kl[kc][:, lc * 128:(lc + 1) * 128],
                                        probs_all[e][kc], start=(kc == 0),
                                        stop=(kc == nkc - 1))
                        for e in (0, 1):
                            h = 2 * pr + e
                            recip = ph.tile([1, 512], F32, name=f"recip{e}",
                                            tag=f"recip{e}", bufs=2)
                            nc.vector.reciprocal(recip, ssums[e])
                            bc_sb = pp.tile([128, 512], F32, name=f"bc_sb{e}",
                                            tag=f"bcsb{e}", bufs=2)
                            nc.gpsimd.partition_broadcast(bc_sb, recip)
                            for i, lc in enumerate((2, 3)):
                                t = pp.tile([128, 512], BF16, name=f"asb{e}b",
                                            tag=f"asb{e}", bufs=5)
                                nc.vector.tensor_copy(t, atts[e][i])
                                asb[e][lc] = t
                            vout = psH.tile([128, 512], F32, name=f"vout{e}",
                                            tag="hmm", bufs=2)
                            for lc in range(LC):
                                nc.tensor.matmul(vout, vc_sbs[e][:, lc, :],
                                                 asb[e][lc],
                                                 start=(lc == 0),
                                                 stop=(lc == LC - 1))
                            nc.vector.tensor_mul(out_v[h][:, sl], vout, bc_sb)

            # =========== phase 6: o_proj partials ===========
            with tc.tile_pool(name="p6", bufs=6) as p6, \
                 tc.tile_pool(name="ps6", bufs=1, space="PSUM") as ps6:
                for dc in range(D // 512):
                    po = [ps6.tile([128, 512], F32, name=f"po{sc2}",
                                   tag=f"po{sc2}", bufs=1)
                          for sc2 in range(SK)]
                    for hv in range(HLOC):
                        ow_t = p6.tile([128, 512], BF16, name="ow_t",
                                       tag="ow_stream")
                        nc.sync.dma_start(
                            ow_t,
                            ow_d[hv * 128:(hv + 1) * 128, dc * 512:(dc + 1) * 512])
                        for sc2 in range(SK):
                            nc.tensor.matmul(
                                po[sc2], out_v[hv][:, sc2 * 128:(sc2 + 1) * 128],
                                ow_t, start=(hv == 0), stop=(hv == HLOC - 1))
                    for sc2 in range(SK):
                        ob = p6.tile([128, 512], F32, name="ob", tag="ob")
                        nc.scalar.copy(ob, po[sc2])
                        nc.sync.dma_start(
                            out_d[sc2 * 128:(sc2 + 1) * 128,
                                  dc * 512:(dc + 1) * 512], ob)

    nc.compile()
    return nc


def host_prep(hidden_states, cos, sin, q_a_w, q_a_ln_w, q_b_w, kv_a_w,
              kv_a_ln_w, kc_w, vc_w, o_w):
    """Shard + preprocess full fp32 inputs into per-core bf16 in_maps."""
    h = np.asarray(hidden_states)[0]            # [S, D]
    hT = np.ascontiguousarray(h.T).astype(bf16)  # [D, S]
    cosT = np.ascontiguousarray(np.asarray(cos).T)  # [64, S]
    sinT = np.ascontiguousarray(np.asarray(sin).T)
    cos2T = np.concatenate([cosT, cosT], 0).astype(bf16)   # [128, S]
    sin2T = np.concatenate([sinT, sinT], 0).astype(bf16)

    # combined col-sharded projection: [q_a | latent]
    lat_w = np.asarray(kv_a_w)[:, :KV_LORA]
    pe_w = np.asarray(kv_a_w)[:, KV_LORA:]
    w_comb = np.concatenate([np.asarray(q_a_w), lat_w], 1)  # [D, 2048]

    # k_pe weight, duplicated rows, [d_in, k, col] layout (replicated)
    pedup = np.concatenate([pe_w, pe_w], 1)                 # [D, 128]
    pew = np.ascontiguousarray(
        pedup.reshape(DK, 128, 128).transpose(1, 0, 2)
        .reshape(128, DK * 128)).astype(bf16)

    # rotate-half permutation (lhsT layout): rot = rot_pt^T @ x
    rot_pt = np.zeros((128, 128), np.float32)
    for b in (0, 1):
        for i in range(32):
            rot_pt[b * 64 + 32 + i, b * 64 + i] = -1.0
            rot_pt[b * 64 + i, b * 64 + 32 + i] = 1.0
    rot_pt = rot_pt.astype(bf16)

    qb = (np.asarray(q_b_w) * np.asarray(q_a_ln_w)[:, None])  # ln folded

    kv_ln = np.asarray(kv_a_ln_w)
    kc_eff = (np.asarray(kc_w) * kv_ln[None, None, :]).astype(bf16)  # [H,128,512]
    vc_eff = (np.asarray(vc_w) * kv_ln[None, :, None]).astype(bf16)  # [H,512,128]
    ow = np.asarray(o_w)

    i = np.arange(128)[:, None]
    j = np.arange(512)[None, :]
    masks = np.stack([(j >= i + off).astype(np.float32)
                      for off in (0, 128, 256, 384)]).astype(bf16)

    hTf = hT.astype(np.float32)  # for hpe slicing (already bf16-rounded)

    in_maps = []
    for c in range(NCORES):
        # my 2 chunks of w_comb, [chunk, d_in, (k, col)] layout
        wcc = w_comb[:, 256 * c: 256 * (c + 1)]
        wc_core = np.ascontiguousarray(
            wcc.reshape(DK, 128, WLOC, 128).transpose(2, 1, 0, 3)
            .reshape(WLOC, 128, D)).astype(bf16)

        # stats selector: col 0 collects q ssq (pre-scaled), col 1 latent
        sel2 = np.zeros((128, 2), np.float32)
        if c < 6:
            sel2[:, 0] = 1.0 / Q_LORA
        else:
            sel2[:, 1] = 1.0 / KV_LORA
        sel2 = sel2.astype(bf16)

        # my seq-slice of hT in [d_in, k, seq] layout
        hpe = np.ascontiguousarray(
            hTf[:, 128 * c:128 * (c + 1)].reshape(DK, 128, 128)
            .transpose(1, 0, 2).reshape(128, DK * 128)).astype(bf16)

        blocks = []
        for p in range(NPAIR):
            h0 = c * HLOC + 2 * p
            h1 = h0 + 1
            n0 = qb[:, h0 * Q_HEAD: h0 * Q_HEAD + NOPE]
            n1 = qb[:, h1 * Q_HEAD: h1 * Q_HEAD + NOPE]
            p0 = qb[:, h0 * Q_HEAD + NOPE: (h0 + 1) * Q_HEAD]
            p1 = qb[:, h1 * Q_HEAD + NOPE: (h1 + 1) * Q_HEAD]
            blocks += [n0, n1, np.concatenate([p0, p1], 1)]
        qb_core0 = np.concatenate(blocks, axis=1).astype(bf16)  # [1536, 3072]
        nqb = QB_COLS // 128
        qb_core = np.ascontiguousarray(
            qb_core0.reshape(QK, 128, nqb, 128).transpose(2, 1, 0, 3)
            .reshape(nqb, 128, Q_LORA))
        in_maps.append({
            "hT": hT,
            "cos2T": cos2T,
            "sin2T": sin2T,
            "wc_w": wc_core,
            "sel2": sel2,
            "hpe": hpe.reshape(128, DK, 128),
            "pew": pew.reshape(128, DK, 128),
            "rot_pt": rot_pt,
            "qb_w": qb_core,
            "kc_w": kc_eff[c * HLOC:(c + 1) * HLOC],
            "vc_w": vc_eff[c * HLOC:(c + 1) * HLOC],
            "o_w": ow[c * OW_ROWS:(c + 1) * OW_ROWS].astype(bf16),
            "masks": masks,
        })
    return in_maps


_CACHE = {}


def _get_program():
    if "nc" not in _CACHE:
        _CACHE["nc"] = build_program()
    return _CACHE["nc"]


def kernel(**inputs) -> np.ndarray:
    from concourse.bass_utils import run_bass_kernel_spmd
    nc = _get_program()
    in_maps = host_prep(**inputs)
    res = run_bass_kernel_spmd(nc, in_maps, list(range(NCORES)))
    out = np.zeros((S, D), np.float64)
    for c in range(NCORES):
        out += res.results[c]["out"].astype(np.float64)
    return out.astype(np.float32)[None]
